# revision 1
# baseline (speedup 1.0000x reference)
"""Trainium2 Bass kernel: ChebWavelet GNN message passing (Chebyshev K=3).

Reference computation:
    T0 = X; T1 = L@X; T2 = 2*L@T1 - T0; out = concat([T0,T1,T2], -1) @ W + b
with L sparse in COO form (edge_row, edge_col, edge_val).

v2 distribution strategy (vs v1's AllGather of T1):
  - Pass 1 (T1 = L@X): DESTINATION-partitioned. Nodes are relabeled by a
    degree-balancing permutation and sharded row-wise over 8 cores; each
    core segment-sums the edges landing in its rows, gathering X source
    rows from a replicated X table (an input, so replication is free).
  - Pass 2 (S = L@T1): SOURCE-partitioned. Each core takes the edges whose
    SOURCE lies in its row slice, so the gather table is its own freshly
    computed T1 slice -- no exchange of T1 at all.  Each core accumulates
    partial sums for ALL destination rows (f-major, grouped by dest core)
    and ReduceScatter(add) -- split in two halves so the first overlaps the
    tail of pass-2 compute -- yields the reduced S rows each core needs.
    This replaces the 16MB AllGather (265us model time) with 2x0.5MB
    reduce-scatters (~28us each, the first hidden under compute).
  - Segment-sum via TensorEngine: per 128-edge chunk, gathered source rows
    (bf16 DMA row gather) are stationary and a {row-offset x edge}
    indicator (VectorE is_equal against an iota, scaled by edge_val) is
    moving, accumulating f-major [64, 512] PSUM tiles (one bank each).
  - Chebyshev recombination 2*L@T1 - T0 is folded into the linear layer:
    out = X@(W0-W2) + T1@W1 + S@(2*W2) + b.
  - Gather tables are stored as packed fp8 (64B rows inside 256B-strided
    table rows): halves the real gather traffic; the gathered messages
    feed the PE as an fp8 stationary operand against the bf16 indicator.
  - Both ReduceScatters issue from the Pool queue after the last gather
    (the only engine allowed to trigger collectives); the lo-half
    reduce-scatter's result is consumed (final linear + output writes for
    the lo tiles) while the hi-half reduce-scatter is still running.
"""

import os

import numpy as np
import ml_dtypes

import concourse.bacc as bacc
import concourse.mybir as mybir
from concourse.bass_types import AP


F = 64          # feature dim
FO = 128        # output feature dim
TILE = 512      # rows per psum tile (one psum bank at f32)
CHUNK = 128     # edges per chunk (PE contraction)
NBUF = 4        # seg psum ring depth
ST2 = 4         # pass-2 tiles per gather/indicator group
MAXCH = 8       # chunks per dma_gather call (1024 descriptors)
SPLIT_RS = bool(int(os.environ.get("SPLIT_RS", "1")))
STOP_ZERO = bool(int(os.environ.get("STOP_ZERO", "1")))
NSCHUNK = 4     # s_sb load chunks


class Cfg:
    def __init__(self, n_nodes, n_edges, n_cores):
        self.N = n_nodes
        self.E = n_edges
        self.NC = n_cores
        self.RPC = n_nodes // n_cores
        self.HALF = n_nodes // 2
        self.T1N = self.RPC // TILE          # pass-1 tiles per core
        self.T2N = n_nodes // TILE           # pass-2 global tiles
        self.TPC2 = self.T2N // n_cores      # pass-2 tiles per dest core
        assert n_nodes % (n_cores * TILE) == 0
        assert self.HALF <= 32768            # int16 gather indices
        assert self.RPC <= 32768
        assert (self.T2N // 2) % ST2 == 0
        assert self.TPC2 % 2 == 0


def _row_chunks(cnt3):
    """Shared fixed-row chunk boundaries per slot type.

    cnt3: [NC, NT, TILE] per-core per-row edge counts.  Walks each slot
    type's rows, closing a chunk when any core's count would exceed CHUNK.
    Returns (K[NT], bounds: list of per-type [(a, b)...], W, rowck[NT,
    TILE] row->chunk-within-type)."""
    NC, NT, T = cnt3.shape
    K = np.zeros(NT, np.int64)
    bounds = []
    rowck = np.zeros((NT, T), np.int64)
    span_max = 1
    for s in range(NT):
        bl = []
        acc = np.zeros(NC, np.int64)
        a = 0
        for row in range(T):
            cr = cnt3[:, s, row]
            if (acc + cr).max() > CHUNK and acc.max() > 0:
                bl.append((a, row))
                span_max = max(span_max, row - a)
                a = row
                acc = cr.copy()
            else:
                acc += cr
            rowck[s, row] = len(bl)
        bl.append((a, T))
        span_max = max(span_max, T - a)
        bounds.append(bl)
        K[s] = len(bl)
    W = max(16, (span_max + 7) // 8 * 8)
    return K, bounds, W, rowck


def _assign(core, styp, rl, K, bounds, W, rowck, NC, E):
    """Chunk id + position per edge from shared row boundaries."""
    cbase = np.concatenate([[0], np.cumsum(K)[:-1]])
    ck = cbase[styp] + rowck[styp, rl]
    key = (core * int(K.sum()) + ck) * (TILE + 1) + rl
    order = np.argsort(key, kind="stable")
    cks = (core * int(K.sum()) + ck)[order]
    uniq, starts = np.unique(cks, return_index=True)
    rank = np.arange(E)
    st = np.zeros(E, np.int64)
    st[starts] = 1
    run_id = np.cumsum(st) - 1
    pos = rank - starts[run_id]
    assert pos.max() < CHUNK
    rbase = np.zeros(int(K.sum()), np.int64)
    for s in range(len(K)):
        for j, (a, b) in enumerate(bounds[s]):
            rbase[cbase[s] + j] = min(a, TILE - W)
    return order, ck[order], pos, rbase, cbase


def _pack_idx(idx_arr):
    """[NC, L] int16 -> [NC, 128, L//16] gather-descriptor layout."""
    NC, L = idx_arr.shape
    w = idx_arr.reshape(NC, L // 16, 16).transpose(0, 2, 1)
    return np.tile(w, (1, 8, 1)).copy()


def _em(a_flat, CH):
    """[NC, CH, CHUNK] -> [NC, 128, CH] chunk-major bf16."""
    t = a_flat.transpose(0, 2, 1)
    return np.ascontiguousarray(t.astype(ml_dtypes.bfloat16))


def _preprocess(X, edge_row, edge_col, edge_val, cfg):
    N, NC, RPC, E = cfg.N, cfg.NC, cfg.RPC, cfg.E
    T1N, T2N, TPC2 = cfg.T1N, cfg.T2N, cfg.TPC2

    deg = np.bincount(edge_row, minlength=N)
    order = np.argsort(-deg, kind="stable")
    G = NC * RPC // 128
    k = np.arange(N)
    rnd, pos = k // G, k % G
    tile_of = np.where(rnd % 2 == 0, pos, G - 1 - pos)
    sigma = np.empty(N, dtype=np.int64)
    sigma[order] = tile_of * 128 + rnd

    r = sigma[edge_row]
    c = sigma[edge_col]
    v = np.asarray(edge_val, dtype=np.float32)

    # ---------------- pass 1: dest-partitioned ----------------
    core1 = r // RPC
    t1 = (r % RPC) // TILE
    h1 = (c >= cfg.HALF).astype(np.int64)
    rl1 = r % TILE
    styp1 = (t1 * 2 + h1).astype(np.int64)          # slot type within core
    cnt3_1 = np.zeros((NC, T1N * 2, TILE), np.int64)
    np.add.at(cnt3_1, (core1, styp1, rl1), 1)
    K1, bounds1, W1, rowck1 = _row_chunks(cnt3_1)
    CH1 = int(K1.sum())
    L1 = CH1 * CHUNK
    o1, ck1, pos1, rbase1, cbase1 = _assign(core1, styp1, rl1, K1, bounds1,
                                            W1, rowck1, NC, E)
    idx1 = np.zeros((NC, L1), np.int16)
    val1 = np.zeros((NC, L1), np.float32)
    rlo1 = np.zeros((NC, L1), np.int32)
    cidx1 = core1[o1]
    p1pos = ck1 * CHUNK + pos1
    idx1[cidx1, p1pos] = (c[o1] - h1[o1] * cfg.HALF).astype(np.int16)
    val1[cidx1, p1pos] = v[o1]
    rlo1[cidx1, p1pos] = rl1[o1] - rbase1[ck1]
    rl1_loc = rlo1.reshape(NC, CH1, CHUNK)
    assert rl1_loc.min() >= 0 and rl1_loc.max() < W1

    # ---------------- pass 2: source-partitioned ----------------
    core2 = c // RPC
    gt = r // TILE                      # global dest tile
    gcore = gt // TPC2                  # dest core
    tt = gt % TPC2                      # tile within dest core
    h2 = (tt >= TPC2 // 2).astype(np.int64)
    o2i = h2 * (T2N // 2) + gcore * (TPC2 // 2) + (tt % (TPC2 // 2))
    rl2 = r % TILE
    cnt3_2 = np.zeros((NC, T2N, TILE), np.int64)
    np.add.at(cnt3_2, (core2, o2i, rl2), 1)
    K2, bounds2, W2, rowck2 = _row_chunks(cnt3_2)
    CH2 = int(K2.sum())
    L2 = CH2 * CHUNK
    o2, ck2, pos2, rbase2, cbase2 = _assign(core2, o2i, rl2, K2, bounds2,
                                            W2, rowck2, NC, E)
    idx2 = np.zeros((NC, L2), np.int16)
    val2 = np.zeros((NC, L2), np.float32)
    rlo2 = np.zeros((NC, L2), np.int32)
    cidx2 = core2[o2]
    p2pos = ck2 * CHUNK + pos2
    idx2[cidx2, p2pos] = (c[o2] % RPC).astype(np.int16)
    val2[cidx2, p2pos] = v[o2]
    rlo2[cidx2, p2pos] = rl2[o2] - rbase2[ck2]
    rl2_loc = rlo2.reshape(NC, CH2, CHUNK)
    assert rl2_loc.min() >= 0 and rl2_loc.max() < W2

    Xp = np.empty((N, F), np.float32)
    Xp[sigma] = np.asarray(X, np.float32)
    xbytes = np.zeros((N, 4 * F), np.uint8)
    xbytes[:, 0:F] = Xp.astype(ml_dtypes.float8_e4m3).view(np.uint8)
    xpad = xbytes.view(ml_dtypes.bfloat16)

    return dict(
        sigma=sigma, Xp=Xp, xpad=xpad,
        K1=K1, cbase1=cbase1, CH1=CH1, W1=W1, rbase1=rbase1,
        idx1_w=_pack_idx(idx1),
        val1_em=_em(val1.reshape(NC, CH1, CHUNK), CH1),
        rloc1_em=_em(rl1_loc, CH1),
        K2=K2, cbase2=cbase2, CH2=CH2, W2=W2, rbase2=rbase2,
        idx2_w=_pack_idx(idx2),
        val2_em=_em(val2.reshape(NC, CH2, CHUNK), CH2),
        rloc2_em=_em(rl2_loc, CH2),
    )


def _gcalls(nch):
    """Split nch chunks into balanced gather calls of <= MAXCH chunks."""
    ncalls = (nch + MAXCH - 1) // MAXCH
    base = nch // ncalls
    rem = nch % ncalls
    out = []
    c0 = 0
    for i in range(ncalls):
        n = base + (1 if i < rem else 0)
        out.append((c0, n))
        c0 += n
    return out


def _raw_collective(eng, kind, op, groups, in_ap, out_ap):
    """collective_compute emission on an arbitrary engine (the bass wrapper
    only exposes it on gpsimd; any engine except sync can trigger one)."""
    from concourse.bass import filter_and_check_groups
    eng.bass.has_collectives = True
    groups = filter_and_check_groups(eng.bass.num_devices, groups)
    return eng.add_instruction(
        mybir.InstCollectiveCompute(
            name=f"I-{eng.bass.next_id()}",
            kind=kind, op=op, replica_groups=groups,
            ins=[eng.lower_ap(in_ap)], outs=[eng.lower_ap(out_ap)],
            unique_tensors="No", cc_dim="Partition",
        ))


def _raw_gather_pap(gp, out_pap, in_ap, idxs_ap, num_idxs, reg, elem_size):
    inst = mybir.InstDMAGatherAnt(
        name=gp.bass.get_next_instruction_name(),
        ins=[*gp.lower_ap_dma(in_ap, for_custom_bir_dma=True),
             gp.lower_ap(idxs_ap),
             gp.lower_val_access(reg)],
        outs=[out_pap],
        transpose=False, num_idxs=num_idxs, elem_size=elem_size,
        stride_bytes_256=1, gen_mode=0, single_packet=True, queue_num=0,
        sbuf_tokens_per_rank=0, sbuf_free_dim_per_rank=0,
        sbuf_free_dim_pad_per_rank=0, sbuf_byte_offset=0,
    )
    return gp.add_instruction(inst)


def _raw_gather(gp, out_ap, in_ap, idxs_ap, num_idxs, reg, elem_size):
    """dma_gather with elem_size_bytes not a multiple of 256 (non-transpose
    gathers take any elem size; table rows must be 256B-strided)."""
    inst = mybir.InstDMAGatherAnt(
        name=gp.bass.get_next_instruction_name(),
        ins=[*gp.lower_ap_dma(in_ap, for_custom_bir_dma=True),
             gp.lower_ap(idxs_ap),
             gp.lower_val_access(reg)],
        outs=[gp.lower_ap(out_ap)],
        transpose=False, num_idxs=num_idxs, elem_size=elem_size,
        stride_bytes_256=1, gen_mode=0, single_packet=True, queue_num=0,
        sbuf_tokens_per_rank=0, sbuf_free_dim_per_rank=0,
        sbuf_free_dim_pad_per_rank=0, sbuf_byte_offset=0,
    )
    return gp.add_instruction(inst)


def _build(cfg, K1, rbase1, W1, K2, rbase2, W2):
    NC, RPC, HALF = cfg.NC, cfg.RPC, cfg.HALF
    T1N, T2N, TPC2 = cfg.T1N, cfg.T2N, cfg.TPC2
    f32, bf16, i16 = mybir.dt.float32, mybir.dt.bfloat16, mybir.dt.int16

    K1 = np.asarray(K1)
    K2 = np.asarray(K2)
    cbase1 = np.concatenate([[0], np.cumsum(K1)[:-1]]).astype(np.int64)
    cbase2 = np.concatenate([[0], np.cumsum(K2)[:-1]]).astype(np.int64)
    CH1, CH2 = int(K1.sum()), int(K2.sum())
    L1, L2 = CH1 * CHUNK, CH2 * CHUNK

    # group structure: pass-1 group = one tile (both halves); pass-2 group =
    # ST2 ordered tiles.  groups share a 3-deep gather+indicator ring.
    NG2 = T2N // ST2
    g1_nch = [int(K1[2 * t] + K1[2 * t + 1]) for t in range(T1N)]
    g2_nch = [int(K2[g * ST2:(g + 1) * ST2].sum()) for g in range(NG2)]
    G1MAX = max(g1_nch)
    G2MAX = max(g2_nch)
    GMAX = max(G1MAX, G2MAX)
    NGRP = T1N + NG2
    WMAX = max(W1, W2)
    RING1 = int(os.environ.get("RING1", "6"))
    RING2 = int(os.environ.get("RING2", "3"))

    # gather calls per group
    def group_calls(grp):
        if grp < T1N:
            t = grp
            calls = []
            for h in range(2):
                cb = int(cbase1[2 * t + h])
                for c0, n in _gcalls(int(K1[2 * t + h])):
                    calls.append(("x", h, cb + c0, n, cb + c0
                                  - int(cbase1[2 * t])))
            return calls
        g = grp - T1N
        calls = []
        gb = int(cbase2[g * ST2])
        for c0, n in _gcalls(g2_nch[g]):
            calls.append(("t", 0, gb + c0, n, c0))
        return calls

    ncalls_grp = [len(group_calls(g)) for g in range(NGRP)]
    def _gsem_idx(g):
        return g % RING1 if g < T1N else RING1 + (g - T1N) % RING2

    cum_calls = [0] * (RING1 + RING2)
    subtot = {}
    for g in range(NGRP):
        cum_calls[_gsem_idx(g)] += ncalls_grp[g]
        subtot[g] = cum_calls[_gsem_idx(g)]

    max_ni = max(n for g in range(NGRP)
                 for (_, _, _, n, _) in group_calls(g)) * CHUNK
    nc = bacc.Bacc(trn_type="TRN2", num_devices=NC,
                   dynamic_dma_scratch_size=max_ni * 16 + 4096)

    # cumulative pass-2 partial-write count (dest-core runs) per group
    p2w_runs = []
    for gr in range(NG2):
        gcs = set()
        for k in range(ST2):
            o = gr * ST2 + k
            j = o % (T2N // 2)
            gcs.add(j // (TPC2 // 2))
        p2w_runs.append(len(gcs))
    p2w_cum = np.cumsum(p2w_runs)

    def _p2w_cum(g):
        return int(p2w_cum[g])

    def _p2w_pcum(g):
        # runs of groups with parity g%2, through group g inclusive
        return int(sum(p2w_runs[i] for i in range(g % 2, g + 1, 2)))

    # ---- dram ----
    fp8 = mybir.dt.float8e4
    xpad = nc.declare_dram_parameter("xpad", [cfg.N, 2 * F], bf16,
                                     isOutput=False)
    xptb = nc.declare_dram_parameter("xptb", [F + 1, RPC], bf16,
                                     isOutput=False)
    idx1 = nc.declare_dram_parameter("idx1", [128, L1 // 16], i16,
                                     isOutput=False)
    val1 = nc.declare_dram_parameter("val1", [128, CH1], bf16, isOutput=False)
    rlc1 = nc.declare_dram_parameter("rlc1", [128, CH1], bf16, isOutput=False)
    idx2 = nc.declare_dram_parameter("idx2", [128, L2 // 16], i16,
                                     isOutput=False)
    val2 = nc.declare_dram_parameter("val2", [128, CH2], bf16, isOutput=False)
    rlc2 = nc.declare_dram_parameter("rlc2", [128, CH2], bf16, isOutput=False)
    iot1 = nc.declare_dram_parameter("iot1", [128, W1 * G1MAX], bf16,
                                     isOutput=False)
    iot2 = nc.declare_dram_parameter("iot2", [128, W2 * G2MAX], bf16,
                                     isOutput=False)
    wb = nc.declare_dram_parameter("wb", [F + 1, 3 * FO], bf16,
                                    isOutput=False)
    out = nc.declare_dram_parameter("out", [RPC, FO], f32, isOutput=True)

    t1tab = nc.dram_tensor("t1tab", [RPC, 2 * F], bf16)

    HRPC = RPC // 2
    if SPLIT_RS:
        part2a = nc.dram_tensor("part2a", [NC * F, HRPC], bf16)
        part2b = nc.dram_tensor("part2b", [NC * F, HRPC], bf16)
        rs_a = nc.dram_tensor("rs_a", [F, HRPC], bf16)
        rs_b = nc.dram_tensor("rs_b", [F, HRPC], bf16)
    else:
        part2f = nc.dram_tensor("part2f", [NC * F, RPC], bf16)
        rs_f = nc.dram_tensor("rs_f", [F, RPC], bf16)

    from contextlib import ExitStack
    with ExitStack() as ctx:
        def sb(name, shape, dt):
            return ctx.enter_context(nc.sbuf_tensor(name, shape, dt))

        def ps(name, shape):
            return ctx.enter_context(
                nc.psum_tensor(name, shape, mybir.dt.float32))

        idx1_sb = sb("idx1_sb", [128, L1 // 16], i16)
        idx2_sb = sb("idx2_sb", [128, L2 // 16], i16)
        val1_sb = sb("val1_sb", [128, CH1], bf16)
        rlc1_sb = sb("rlc1_sb", [128, CH1], bf16)
        val2_sb = sb("val2_sb", [128, CH2], bf16)
        rlc2_sb = sb("rlc2_sb", [128, CH2], bf16)
        iot1_sb = sb("iot1_sb", [128, W1 * G1MAX], bf16)
        iot2_sb = sb("iot2_sb", [128, W2 * G2MAX], bf16)
        xptb_sb = sb("xptb_sb", [F + 1, RPC], bf16)
        wb_sb = sb("wb_sb", [F + 1, 3 * FO], bf16)
        zero_sb = sb("zero_sb", [128, TILE], bf16)
        dst_f8 = sb("dst_f8", [128, RING1 * G1MAX * F], fp8)
        dst_sb = sb("dst_sb", [128, RING2 * G2MAX * F], bf16)
        ind1_sb = sb("ind1_sb", [128, RING1 * G1MAX * W1], bf16)
        ind2_sb = sb("ind2_sb", [128, RING2 * G2MAX * W2], bf16)
        t1fb_sb = sb("t1fb_sb", [F, RPC], bf16)
        t1row_sb = sb("t1row_sb", [128, 2 * (TILE // 128) * F], bf16)
        part_sb = sb("part_sb", [F, 2 * ST2 * TILE], bf16)   # 2-group ring
        s_sb = sb("s_sb", [F, RPC], bf16)
        out_sb = sb("out_sb", [128, 8 * FO], f32)

        seg_ps = [ps(f"seg_ps{i}", [F, TILE]) for i in range(NBUF)]
        out_ps = [ps(f"out_ps{i}", [128, FO]) for i in range(4)]

        sem = {name: ctx.enter_context(nc.semaphore(name)) for name in
               ["ldA", "ldB", "ldC", "ldD", "ldE", "ldF",
                "sc", "seg", "ev",
                "trv0", "trv1", "t1w0", "t1w1", "p2w0", "p2w1",
                "ccA", "ccB", "sA", "sB", "o", "oev", "ow0", "ow1", "zz"]}
        gsems = [ctx.enter_context(nc.semaphore(f"g{i}"))
                 for i in range(RING1 + RING2)]

        TPT = TILE // 128               # transpose slices per pass-1 tile
        NE1 = T1N                       # pass-1 psum tiles
        NE2 = T2N                       # pass-2 psum tiles
        NET = NE1 + NE2

        def tiles_thru(grp):
            return grp + 1 if grp < T1N else NE1 + (grp - T1N + 1) * ST2

        def seg_war_wait(eng, ei):
            pei = ei - NBUF
            if pei >= 0:
                eng.wait_ge(sem["ev"], pei + 1)

        # ---------------- SP: loads + writes ----------------
        with nc.Block() as block:

            @block.sync
            def _(sync):
                H1 = (L1 // 16) // 2
                for dst, src, sname in [
                    (idx1_sb[:, 0:H1], idx1[:, 0:H1], "ldA"),
                    (val1_sb[:, :], val1[:, :], "ldB"),
                    (rlc1_sb[:, :], rlc1[:, :], "ldB"),
                    (iot1_sb[:, :], iot1[:, :], "ldB"),
                    (None, None, "IDX1H2"),
                    (idx2_sb[:, :], idx2[:, :], "ldD"),
                    (val2_sb[:, :], val2[:, :], "ldE"),
                    (rlc2_sb[:, :], rlc2[:, :], "ldE"),
                    (iot2_sb[:, :], iot2[:, :], "ldE"),
                    (xptb_sb[:, :], xptb[:, :], "ldF"),
                    (wb_sb[:, :], wb[:, :], "ldF"),
                ]:
                    if sname == "IDX1H2":
                        sync.wait_ge(sem["ldA"], 16)
                        sync.dma_start(idx1_sb[:, H1:], idx1[:, H1:]
                                       ).then_inc(sem["ldA"], 16)
                        continue
                    sync.dma_start(dst, src).then_inc(sem[sname], 16)
                # t1 transposes (xbar DMA) + t1tab writes, per pass-1 tile
                for t in range(T1N):
                    b = t % 2
                    trv = sem["trv0" if b == 0 else "trv1"]
                    sync.wait_ge(sem["ev"], t + 1)
                    if t >= 2:
                        sync.wait_ge(sem["t1w0" if b == 0 else "t1w1"],
                                     16 * (t // 2))
                    for sl in range(TPT):
                        sync.dma_start_transpose(
                            t1row_sb[:, (b * TPT + sl) * F:
                                     (b * TPT + sl + 1) * F],
                            t1fb_sb[:, t * TILE + sl * 128:
                                    t * TILE + (sl + 1) * 128],
                        ).then_inc(trv, 16)
                    sync.wait_ge(trv, 16 * (t // 2 + 1) * TPT)
                    src = t1row_sb[:, b * TPT * F:(b + 1) * TPT * F]
                    src3 = AP(src.tensor, src.offset,
                              [src.ap[0], [F, TPT], [1, F]])
                    dst = t1tab[t * TILE:(t + 1) * TILE, 0:F]
                    dst3 = AP(dst.tensor, dst.offset,
                              [[2 * F, 128], [2 * F * 128, TPT], [1, F]])
                    sync.dma_start(dst3, src3).then_inc(
                        sem["t1w0" if t % 2 == 0 else "t1w1"], 16)
                # pass-2 partial writes: one per (group, dest-core run)
                for gr in range(NG2):
                    b = gr % 2
                    sync.wait_ge(sem["ev"], NE1 + (gr + 1) * ST2)
                    h2g = 1 if (gr * ST2) >= (T2N // 2) else 0
                    runs = {}
                    for k in range(ST2):
                        o = gr * ST2 + k
                        j = o % (T2N // 2)
                        gc = j // (TPC2 // 2)
                        runs.setdefault(gc, []).append(k)
                    for gc, ks in runs.items():
                        k0 = ks[0]
                        o0 = gr * ST2 + k0
                        j0 = o0 % (T2N // 2)
                        if SPLIT_RS:
                            part2x = part2a if h2g == 0 else part2b
                            c0 = (j0 % (TPC2 // 2)) * TILE
                        else:
                            part2x = part2f
                            tt0 = h2g * (TPC2 // 2) + (j0 % (TPC2 // 2))
                            c0 = tt0 * TILE
                        src = part_sb[:, (b * ST2 + k0) * TILE:
                                      (b * ST2 + k0 + len(ks)) * TILE]
                        dst = part2x[gc * F:(gc + 1) * F,
                                     c0:c0 + len(ks) * TILE]
                        sync.dma_start(dst, src).then_inc(
                            sem["p2w0" if gr % 2 == 0 else "p2w1"], 16)
                # S slice loads after the reduce-scatter(s)
                NOT = RPC // 128

                def out_write(j):
                    sync.wait_ge(sem["oev"], 4 * (j + 1))
                    src = out_sb[:, (j % 2) * 4 * FO:((j % 2) * 4 + 4) * FO]
                    src3 = AP(src.tensor, src.offset,
                              [src.ap[0], [FO, 4], [1, FO]])
                    dst = out[j * 512:(j + 1) * 512, :]
                    dst3 = AP(dst.tensor, dst.offset,
                              [[FO, 128], [FO * 128, 4], [1, FO]])
                    sync.dma_start(dst3, src3).then_inc(
                        sem["ow0" if j % 2 == 0 else "ow1"], 16)

                if SPLIT_RS:
                    QW = HRPC // 2
                    sync.wait_ge(sem["ccA"], 1)
                    for q in range(2):
                        if q:
                            sync.wait_ge(sem["sA"], 16 * q)
                        sync.dma_start(s_sb[:, q * QW:(q + 1) * QW],
                                       rs_a[:, q * QW:(q + 1) * QW]
                                       ).then_inc(sem["sA"], 16)
                    for j in range(NOT // 8):
                        out_write(j)
                    sync.wait_ge(sem["ccB"], 1)
                    for q in range(2):
                        if q:
                            sync.wait_ge(sem["sB"], 16 * q)
                        sync.dma_start(s_sb[:, HRPC + q * QW:
                                            HRPC + (q + 1) * QW],
                                       rs_b[:, q * QW:(q + 1) * QW]
                                       ).then_inc(sem["sB"], 16)
                    for j in range(NOT // 8, NOT // 4):
                        out_write(j)
                else:
                    sync.wait_ge(sem["ccB"], 1)
                    CW = RPC // NSCHUNK
                    for q in range(NSCHUNK):
                        sync.dma_start(
                            s_sb[:, q * CW:(q + 1) * CW],
                            rs_f[:, q * CW:(q + 1) * CW],
                        ).then_inc(sem["sA" if q % 2 == 0 else "sB"], 16)
                    for j in range(NOT // 4):
                        out_write(j)


            # ---------------- gpsimd: gathers ----------------
            @block.gpsimd
            def _(gp):
                ni_regs = {}
                for g in range(NGRP):
                    for (_, _, _, n, _) in group_calls(g):
                        ni = n * CHUNK
                        if ni not in ni_regs:
                            ni_regs[ni] = gp.to_reg(ni)
                pf = gp.lower_ap(dst_f8[:, 0:4])
                np_part = RING1 * G1MAX * F // 4
                pap32 = mybir.PhysicalAccessPattern(
                    ap=[[np_part, 128], [1, np_part]], offset=0,
                    dtype=mybir.dt.float32, memref=pf.memref,
                    memsetref=pf.memsetref, bass_ap=dst_f8[:, :])
                gp.add_instruction(mybir.InstMemset(
                    name=f"I-{gp.bass.next_id()}",
                    mode="Const", constant=0, ins=[], outs=[pap32]))
                gp.drain()
                gp.wait_ge(sem["ldA"], 16)          # idx1 first half
                H1g = (L1 // 16) // 2
                idx1_full_wait_done = False
                for grp in range(NGRP):
                    b = grp % RING1 if grp < T1N else (grp - T1N) % RING2
                    bs = _gsem_idx(grp)
                    if (grp < T1N and not idx1_full_wait_done
                            and (int(cbase1[2 * grp + 1])
                                 + int(K1[2 * grp + 1])) * 8 > H1g):
                        gp.wait_ge(sem["ldA"], 32)      # idx1 second half
                        idx1_full_wait_done = True
                    if grp < T1N:
                        if grp >= RING1:
                            gp.wait_ge(sem["seg"], tiles_thru(grp - RING1))
                    elif grp - T1N >= RING2:
                        gp.wait_ge(sem["seg"], tiles_thru(grp - RING2))
                    if grp == T1N:
                        gp.wait_ge(sem["ldD"], 16)      # idx2
                        gp.wait_ge(sem["t1w0"], 16 * (T1N // 2))
                        gp.wait_ge(sem["t1w1"], 16 * (T1N - T1N // 2))
                    for (tab, h, cks, n, off) in group_calls(grp):
                        ni = n * CHUNK
                        FH = F // 2
                        if tab == "x":
                            # fp8-packed table read as 32 bf16 cells/row;
                            # out pap: bf16 view over the fp8 message ring
                            table = xpad[h * HALF:(h + 1) * HALF, 0:FH]
                            bofs = b * G1MAX * FH + off * FH
                            probe = gp.lower_ap(dst_f8[:, 0:1])
                            out_pap = mybir.PhysicalAccessPattern(
                                ap=[[RING1 * G1MAX * FH, 128], [FH, n], [1, FH]],
                                offset=bofs, dtype=bf16,
                                memref=probe.memref,
                                memsetref=probe.memsetref,
                                bass_ap=dst_f8[:, 2 * bofs:
                                               2 * (bofs + n * FH)],
                            )
                            _raw_gather_pap(
                                gp, out_pap, table,
                                idx1_sb[:, cks * 8:cks * 8 + ni // 16],
                                ni, ni_regs[ni], FH,
                            ).then_inc(gsems[bs], 16)
                        else:
                            table = t1tab[:, 0:F]
                            dst = dst_sb[:, b * G2MAX * F + off * F:
                                         b * G2MAX * F + (off + n) * F]
                            dst3 = AP(dst.tensor, dst.offset,
                                      [dst.ap[0], [F, n], [1, F]])
                            _raw_gather(
                                gp, dst3, table,
                                idx2_sb[:, cks * 8:cks * 8 + ni // 16],
                                ni, ni_regs[ni], F,
                            ).then_inc(gsems[bs], 16)
                # reduce-scatter(s) on the Pool queue once gathers are done
                if SPLIT_RS:
                    gp.wait_ge(sem["p2w0"],
                               16 * _p2w_pcum(NG2 // 2 - 1
                                              - (NG2 // 2 - 1) % 2))
                    gp.wait_ge(sem["p2w1"],
                               16 * _p2w_pcum(NG2 // 2 - 1 - (NG2 // 2) % 2))
                    _raw_collective(
                        gp, "ReduceScatter", mybir.AluOpType.add,
                        [list(range(NC))], part2a[:, :], rs_a[:, :],
                    ).then_inc(sem["ccA"], 1)
                gp.wait_ge(sem["p2w0"], 16 * _p2w_pcum(NG2 - 1
                                                       - (NG2 - 1) % 2))
                gp.wait_ge(sem["p2w1"], 16 * _p2w_pcum(NG2 - 1 - NG2 % 2))
                _raw_collective(
                    gp, "ReduceScatter", mybir.AluOpType.add,
                    [list(range(NC))],
                    (part2b if SPLIT_RS else part2f)[:, :],
                    (rs_b if SPLIT_RS else rs_f)[:, :],
                ).then_inc(sem["ccB"], 1)

            # ---------------- vector: indicators + bias ----------------
            @block.vector
            def _(ve):
                ve.memset(zero_sb[:, :], 0.0)
                ve.drain()
                ve.sem_inc(sem["zz"], 1)
                ve.wait_ge(sem["ldB"], 48)          # val1,rlc1,iot1
                for grp in range(NGRP):
                    b = grp % RING1 if grp < T1N else (grp - T1N) % RING2
                    if grp == T1N:
                        ve.wait_ge(sem["ldE"], 48)      # val2,rlc2,iot2
                    if grp < T1N:
                        if grp >= RING1:
                            ve.wait_ge(sem["seg"], tiles_thru(grp - RING1))
                    elif grp - T1N >= RING2:
                        ve.wait_ge(sem["seg"], tiles_thru(grp - RING2))
                    if grp < T1N:
                        nch = g1_nch[grp]
                        W = W1
                        gmx = G1MAX
                        gb = int(cbase1[2 * grp])
                        rlx, vlx, iox = rlc1_sb, val1_sb, iot1_sb
                        ind_ring = ind1_sb
                    else:
                        g = grp - T1N
                        nch = g2_nch[g]
                        W = W2
                        gmx = G2MAX
                        gb = int(cbase2[g * ST2])
                        rlx, vlx, iox = rlc2_sb, val2_sb, iot2_sb
                        ind_ring = ind2_sb
                    ind = ind_ring[:, b * gmx * W:b * gmx * W + nch * W]
                    ind3 = AP(ind.tensor, ind.offset,
                              [ind.ap[0], [nch, W], [1, nch]])
                    rl = rlx[:, gb:gb + nch]
                    rl3 = AP(rl.tensor, rl.offset,
                             [rl.ap[0], [0, W], [1, nch]])
                    io = iox[:, :]
                    io3 = AP(io.tensor, io.offset,
                             [io.ap[0], [gmx, W], [1, nch]])
                    ve.tensor_tensor(ind3, rl3, io3, mybir.AluOpType.is_equal)
                    ve.drain()
                    vl = vlx[:, gb:gb + nch]
                    vl3 = AP(vl.tensor, vl.offset,
                             [vl.ap[0], [0, W], [1, nch]])
                    ve.tensor_tensor(ind3, ind3, vl3, mybir.AluOpType.mult
                                     ).then_inc(sem["sc"], 1)


            # ---------------- scalar: psum evacuations ----------------
            @block.scalar
            def _(sc):
                for t in range(T1N):
                    ei = t
                    b = ei % NBUF
                    sc.wait_ge(sem["seg"], ei + 1)
                    sc.activation(
                        t1fb_sb[:, t * TILE:(t + 1) * TILE],
                        seg_ps[b][:, :],
                        mybir.ActivationFunctionType.Copy,
                    ).then_inc(sem["ev"], 1)
                for g in range(NG2):
                    bg = g % 2
                    for k in range(ST2):
                        ei = NE1 + g * ST2 + k
                        b = ei % NBUF
                        sc.wait_ge(sem["seg"], ei + 1)
                        # WAR on part ring: writes of group g-2 done
                        if g >= 2:
                            sc.wait_ge(sem["p2w0" if g % 2 == 0 else "p2w1"],
                                       16 * _p2w_pcum(g - 2))
                        sc.activation(
                            part_sb[:, (bg * ST2 + k) * TILE:
                                    (bg * ST2 + k + 1) * TILE],
                            seg_ps[b][:, :],
                            mybir.ActivationFunctionType.Copy,
                        ).then_inc(sem["ev"], 1)
                # final out evacuations
                NOT = RPC // 128
                for ot in range(NOT):
                    b2 = ot % 4
                    b8 = ot % 8
                    sc.wait_ge(sem["o"], ot + 1)
                    if ot >= 8:
                        j = (ot - 8) // 4
                        sc.wait_ge(sem["ow0" if j % 2 == 0 else "ow1"],
                                   16 * (j // 2 + 1))
                    sc.activation(out_sb[:, b8 * FO:(b8 + 1) * FO],
                                  out_ps[b2][:, :],
                                  mybir.ActivationFunctionType.Copy,
                                  ).then_inc(sem["oev"], 1)

            # ---------------- tensor: matmuls + collectives ----------------
            @block.tensor
            def _(te):
                te.wait_ge(sem["zz"], 1)
                for grp in range(NGRP):
                    b3 = grp % RING1 if grp < T1N else (grp - T1N) % RING2
                    gmx3 = G1MAX if grp < T1N else G2MAX
                    wx3 = W1 if grp < T1N else W2
                    dbase = b3 * gmx3 * F
                    ibase = b3 * gmx3 * wx3
                    te.wait_ge(sem["sc"], grp + 1)
                    te.wait_ge(gsems[_gsem_idx(grp)], 16 * subtot[grp])
                    if grp < T1N:
                        tiles = [grp]
                        W = W1
                        gb = int(cbase1[2 * grp])
                    else:
                        g = grp - T1N
                        tiles = list(range(g * ST2, (g + 1) * ST2))
                        W = W2
                        gb = int(cbase2[g * ST2])
                    nch_g = g1_nch[grp] if grp < T1N else g2_nch[grp - T1N]
                    for k, tl in enumerate(tiles):
                        ei = tl if grp < T1N else NE1 + tl
                        b = ei % NBUF
                        seg_war_wait(te, ei)
                        te.matmul(seg_ps[b][:, :], zero_sb[:, 0:F],
                                  zero_sb[:, :], start=True, stop=False)
                        if grp < T1N:
                            cklist = [(int(cbase1[2 * grp + h]) + j, rbase1)
                                      for h in range(2)
                                      for j in range(int(K1[2 * grp + h]))]
                        else:
                            cklist = [(int(cbase2[tl]) + j, rbase2)
                                      for j in range(int(K2[tl]))]
                        dring = dst_f8 if grp < T1N else dst_sb
                        for ci, (ck, rbasex) in enumerate(cklist):
                            rb = int(rbasex[ck])
                            off = ck - gb
                            stat = dring[:, dbase + off * F:
                                         dbase + (off + 1) * F]
                            iring = ind1_sb if grp < T1N else ind2_sb
                            ind = iring[:, ibase + off:ibase + off + 1]
                            ind3 = AP(ind.tensor, ind.offset,
                                      [ind.ap[0], [nch_g, W]])
                            last = (not STOP_ZERO) and ci == len(cklist) - 1
                            mm = te.matmul(seg_ps[b][:, rb:rb + W], stat,
                                           ind3, start=False, stop=last,
                                           skip_group_check=True)
                        if STOP_ZERO:
                            te.matmul(seg_ps[b][:, :], zero_sb[:, 0:F],
                                      zero_sb[:, :], start=False, stop=True,
                                      ).then_inc(sem["seg"], 1)
                        else:
                            mm.then_inc(sem["seg"], 1)
                # final linear for the lo half (overlaps reduce-scatter B),
                # then reduce-scatter B, then the hi half
                NOT = RPC // 128

                def final_tile(ot):
                    b2 = ot % 4
                    if SPLIT_RS:
                        half = 0 if ot < NOT // 2 else 1
                        q = (ot % (NOT // 2)) // (NOT // 4)
                        te.wait_ge(sem["sA" if half == 0 else "sB"],
                                   16 * (q + 1))
                    else:
                        q = (ot * 128) // (RPC // NSCHUNK)
                        te.wait_ge(sem["sA" if q % 2 == 0 else "sB"],
                                   16 * (q // 2 + 1))
                    if ot >= 4:
                        te.wait_ge(sem["oev"], ot - 3)
                    hs = [
                        xptb_sb[:, ot * 128:(ot + 1) * 128],
                        t1fb_sb[:, ot * 128:(ot + 1) * 128],
                        s_sb[:, ot * 128:(ot + 1) * 128],
                    ]
                    for j in range(3):
                        mov = (wb_sb[0:F + 1, 0:FO] if j == 0
                               else wb_sb[0:F, j * FO:(j + 1) * FO])
                        mm = te.matmul(
                            out_ps[b2][:, :], hs[j], mov,
                            start=(j == 0), stop=(j == 2),
                        )
                    mm.then_inc(sem["o"], 1)

                te.wait_ge(sem["ldF"], 32)          # xptb, wb
                for ot in range(NOT):
                    final_tile(ot)

    if not nc.is_finalized():
        nc.finalize()
    return nc


_CACHE = {}


def _get_program(cfg, prep):
    key = (cfg.N, cfg.E, cfg.NC,
           prep["K1"].tobytes(), prep["W1"], prep["rbase1"].tobytes(),
           prep["K2"].tobytes(), prep["W2"], prep["rbase2"].tobytes())
    if key not in _CACHE:
        _CACHE[key] = _build(cfg, prep["K1"], prep["rbase1"], prep["W1"],
                             prep["K2"], prep["rbase2"], prep["W2"])
    return _CACHE[key]


def _make_in_maps(prep, W_mat, b, cfg):
    W1, W2 = prep["W1"], prep["W2"]
    K1, K2 = prep["K1"], prep["K2"]
    g1_nch = [int(K1[2 * t] + K1[2 * t + 1]) for t in range(cfg.T1N)]
    NG2 = cfg.T2N // ST2
    g2_nch = [int(K2[g * ST2:(g + 1) * ST2].sum()) for g in range(NG2)]
    G1MAX = max(g1_nch)
    G2MAX = max(g2_nch)
    W_mat = np.asarray(W_mat, np.float32)
    b = np.asarray(b, np.float32)
    W0, W1m, W2m = W_mat[:F], W_mat[F:2 * F], W_mat[2 * F:]
    wb = np.zeros((F + 1, 3 * FO), np.float32)
    wb[:F] = np.concatenate([W0 - W2m, W1m, 2.0 * W2m], axis=1)
    wb[F, 0:FO] = b
    wb = wb.astype(ml_dtypes.bfloat16)
    iot1 = np.broadcast_to(
        np.repeat(np.arange(W1, dtype=np.float32), G1MAX),
        (128, W1 * G1MAX)).astype(ml_dtypes.bfloat16).copy()
    iot2 = np.broadcast_to(
        np.repeat(np.arange(W2, dtype=np.float32), G2MAX),
        (128, W2 * G2MAX)).astype(ml_dtypes.bfloat16).copy()
    Xp = prep["Xp"]
    in_maps = []
    for c in range(cfg.NC):
        xptb = np.ones((F + 1, cfg.RPC), np.float32)
        xptb[:F] = Xp[c * cfg.RPC:(c + 1) * cfg.RPC].T
        xptb = np.ascontiguousarray(xptb.astype(ml_dtypes.bfloat16))
        in_maps.append(dict(
            xpad=prep["xpad"], xptb=xptb,
            idx1=prep["idx1_w"][c], val1=prep["val1_em"][c],
            rlc1=prep["rloc1_em"][c],
            idx2=prep["idx2_w"][c], val2=prep["val2_em"][c],
            rlc2=prep["rloc2_em"][c],
            iot1=iot1, iot2=iot2, wb=wb,
        ))
    return in_maps


def kernel(X, edge_row, edge_col, edge_val, W, b):
    X = np.asarray(X, np.float32)
    edge_row = np.asarray(edge_row, np.int32)
    edge_col = np.asarray(edge_col, np.int32)
    edge_val = np.asarray(edge_val, np.float32)
    cfg = Cfg(X.shape[0], edge_row.shape[0], 8)
    prep = _preprocess(X, edge_row, edge_col, edge_val, cfg)
    nc = _get_program(cfg, prep)
    in_maps = _make_in_maps(prep, W, b, cfg)

    from concourse.bass_utils import run_bass_kernel_spmd
    res = run_bass_kernel_spmd(nc, in_maps, list(range(cfg.NC)))
    out_perm = np.concatenate([res.results[i]["out"] for i in range(cfg.NC)],
                              axis=0)
    return np.ascontiguousarray(out_perm[prep["sigma"]]).astype(np.float32)

